# revision 23
# baseline (speedup 1.0000x reference)
"""APPNP GNN kernel for 8 TRN2 NeuronCores — gather + on-chip reduce, v3.

Reference computation (N=100000 nodes, E=1600000 edges, K=5, alpha=0.5):
    h0 = x @ W1 + b1
    deg[d] = |in-edges(d)| + 1 (self loop); dinv = rsqrt(deg)
    5x: h = (1-a) * dinv * S(dinv * h) + a * h0     (S = adjacency sum + self)
    out = relu(h) @ W2 + b2

v3 structure (per core, nodes row-sharded 12500/core padded to 12544):
  Node rows are remapped to a physical layout with a zero row at 6271 and
  12501.. so each HALF (rows [0,6272) / [6272,12544)) contains a zero row.
  g_full is split as TWO shared tensors gf0/gf1 (one per source half), each
  all-gathered separately so the half-collectives pipeline under compute:
    chunk = 2*half + quad   (quad = src cores 0-3 / 4-7), 25088 rows each.
  Iteration: phase A gathers (chunks 0,1; needs only gf0) -> phase B
  (chunks 2,3; needs gf1) -> partial write-outs -> L2 unsort-gathers
  (region A = chunks {0,1}, B = {2,3}) -> DVE combine + g update per
  7-tile group; when tiles 0-48 are updated, cc0 is written and gf0's
  AllGather for the next iteration fires (while tiles 49-97 still combine),
  then cc1/gf1.  Desc-gen on the Pool engine (~2.3ns/desc serialized) is
  the bottleneck; everything else hides behind it.
  lin1 runs in bf16 (xT + W1 staged bf16).  Epilogue is batched by 7-tile
  groups (one semaphore hop per group, ops back-to-back per engine):
  u = relu(dinvh*msum + a*h0); out = u @ W2 + b2 (PE transpose).
  10 gather slots (k%10), queue k%4, 48KB SWDGE ring; region-A L2 gathers
  are spliced into phase B so only region B sits on the iteration tail.
  lin1 xT streams in 768-col bf16 groups.
  Measured: 4.313ms HW (baseline 4.652ms), rel err 2.6e-3.
"""

import math
import numpy as np

# ----------------------------------------------------------------- config

class Cfg:
    def __init__(self, N=100000, E=1600000, F=500, H=64, O=40, K=5, alpha=0.5,
                 cores=8, batch_slots=4096):
        self.N, self.E, self.F, self.H, self.O, self.K = N, E, F, H, O, K
        self.alpha = alpha
        self.cores = cores
        assert N % cores == 0
        self.NP = N // cores                      # real nodes per core
        self.PL = ((self.NP + 127) // 128) * 128  # padded nodes per core
        self.T = self.PL // 128                   # tiles per core
        self.HALF = self.PL // 2                  # rows per half (6272)
        self.TH = self.T // 2                     # tiles per half (49)
        self.FP = ((F + 127) // 128) * 128        # padded feature dim
        self.KT = self.FP // 128                  # k tiles for lin1
        self.NCH = 4                              # chunks: (half, quad)
        self.CH = cores // 2 * self.HALF          # chunk rows (25088)
        assert self.CH <= 32600
        self.DMA_SCRATCH = 49152                  # SWDGE ring carveout bytes
        self.NQ = 4                               # SWDGE queues
        self.NS = 10                              # gather slots (k % NS)
        self.SLOTC = 15                           # columns per gather slot
        self.BS = batch_slots                     # max slots per gather call
        self.TG = 7                               # level-2 tile group (49%7==0)
        self.LS = 2                               # level-2 slots per region
        self.SP = False                           # single_packet for gathers

FULL = Cfg()


def _phys_map(cfg):
    """real local index (0..NP-1) -> physical row (zero rows at HALF-1 and
    NP+1..PL-1 physical positions)."""
    a = np.arange(cfg.NP, dtype=np.int64)
    return np.where(a < cfg.HALF - 1, a, a + 1)

# ----------------------------------------------------------- host preprocess

def _wrap16(arr):
    w = arr.reshape(-1, 16).T.astype(np.int16)       # [16, S/16]
    return np.ascontiguousarray(np.tile(w, (8, 1)))  # [128, S/16]


def build_plan(cfg, edge_index):
    """Shared (core-independent) call structure + per-core index tables."""
    N, PL, NP, CH, HALF = cfg.N, cfg.PL, cfg.NP, cfg.CH, cfg.HALF
    src = np.asarray(edge_index[0], dtype=np.int64)
    dst = np.asarray(edge_index[1], dtype=np.int64)
    ph = _phys_map(cfg)

    core_of_dst = dst // NP
    dst_loc = ph[dst % NP]                        # physical row of dst
    core_s = src // NP
    loc_s = ph[src % NP]                          # physical row of src
    half_s = loc_s // HALF
    chunk = 2 * half_s + core_s // (cfg.cores // 2)
    src_loc = (core_s % (cfg.cores // 2)) * HALF + (loc_s % HALF)

    # per (core, chunk, dst) degrees
    key = (core_of_dst * cfg.NCH + chunk) * PL + dst_loc
    counts = np.bincount(key, minlength=cfg.cores * cfg.NCH * PL)
    d = counts.reshape(cfg.cores, cfg.NCH, PL)

    # degree-descending order per (core, chunk); pos = rank of node
    ordr = np.argsort(-d, axis=2, kind="stable")           # [co, c, PL]
    pos = np.empty_like(ordr)
    ar = np.arange(PL)
    for co in range(cfg.cores):
        for c in range(cfg.NCH):
            pos[co, c, ordr[co, c]] = ar

    # shared per-tile degree: max over cores of tile-max (desc sort -> first)
    dsort = np.take_along_axis(d, ordr, axis=2)            # [co, c, PL] desc
    Dtile = dsort[:, :, ::128].max(axis=0)                 # [c, T]
    Dtile = np.maximum(Dtile, 1)
    maxD = int(Dtile.max())
    assert maxD <= cfg.SLOTC, f"tile degree {maxD} exceeds slot cols"

    # main gather calls: runs of equal-D tiles, <= SLOTC cols
    calls = []           # (chunk, col_off, ncols, D, nt, t0)
    colbase = np.zeros((cfg.NCH, cfg.T), dtype=np.int64)
    col = 0
    for c in range(cfg.NCH):
        t = 0
        while t < cfg.T:
            D = int(Dtile[c, t])
            nt = 1
            while (t + nt < cfg.T and int(Dtile[c, t + nt]) == D
                   and (nt + 1) * D <= cfg.SLOTC
                   and (nt + 1) * D * 128 <= cfg.BS):
                nt += 1
            colbase[c, t:t + nt] = col + np.arange(nt) * D
            calls.append((c, col, nt * D, D, nt, t))
            col += nt * D
            t += nt
    Ctot = col
    S1 = 128 * Ctot

    # per-core main index tables
    o1 = np.argsort(key, kind="stable")
    ks = key[o1]
    first = np.r_[True, ks[1:] != ks[:-1]]
    starts = np.where(first, np.arange(len(ks)), 0)
    occ = np.arange(len(ks)) - np.maximum.accumulate(starts)
    occ_e = np.empty_like(occ)
    occ_e[o1] = occ

    ZR = HALF - 1                                 # zero row in every chunk
    gidx_all = []
    rank_e = pos[core_of_dst, chunk, dst_loc]     # rank of dst in (co,chunk)
    t_e = rank_e // 128
    p_e = rank_e % 128
    slot_e = (colbase[chunk, t_e] + occ_e) * 128 + p_e
    for co in range(cfg.cores):
        flat = np.full(S1, ZR, dtype=np.int16)
        m = core_of_dst == co
        flat[slot_e[m]] = src_loc[m].astype(np.int16)
        gidx_all.append(_wrap16(flat))

    # level-2: region r, natural tile t, k in {0,1} -> chunk 2r+k
    L2N = 2 * cfg.T * 2 * 128                     # slots over both regions
    l2_calls = []
    for r in range(2):
        for t0 in range(0, cfg.T, cfg.TG):
            nt = min(cfg.TG, cfg.T - t0)
            l2_calls.append((r, t0, nt, r * L2N // 2 + t0 * 2 * 128))
    lidx_all = []
    nodes = np.arange(PL)
    for co in range(cfg.cores):
        flat = np.empty(L2N, dtype=np.int16)
        for r in range(2):
            for k in range(2):
                v = (k * PL + pos[co, 2 * r + k, nodes]).astype(np.int16)
                sl = ((nodes // 128) * 2 + k) * 128 + nodes % 128
                flat[r * (L2N // 2) + sl] = v
        lidx_all.append(_wrap16(flat))

    class Plan:
        pass
    plan = Plan()
    plan.calls = calls
    plan.l2 = l2_calls
    plan.S1 = S1
    plan.L2N = L2N
    plan.maxD = maxD
    plan.pad_frac = (S1 - int(np.sum(d)) / cfg.cores) / max(S1, 1)
    return plan, gidx_all, lidx_all


def host_prep(cfg, x, edge_index, W1, b1, W2, b2):
    import ml_dtypes
    bf16 = ml_dtypes.bfloat16
    N, H, F = cfg.N, cfg.H, cfg.F
    dst = np.asarray(edge_index[1], dtype=np.int64)
    deg = np.bincount(dst, minlength=N).astype(np.float64) + 1.0
    dinv = (1.0 / np.sqrt(deg)).astype(np.float32)
    ph = _phys_map(cfg)

    plan, gidx_all, lidx_all = build_plan(cfg, edge_index)

    def tileize(v):  # [PL] -> [128, T]
        return np.ascontiguousarray(v.reshape(cfg.T, 128).T)

    xT = np.zeros((cfg.FP, cfg.PL), dtype=bf16)
    W1p = np.zeros((cfg.FP, H), dtype=bf16)
    W1p[:F] = W1.astype(bf16)
    b1r = np.ascontiguousarray(np.broadcast_to(b1.astype(np.float32), (128, H)))
    b2r = np.ascontiguousarray(np.broadcast_to(b2.astype(np.float32), (128, cfg.O)))
    ident = np.eye(128, dtype=np.float32)

    in_maps = []
    for c in range(cfg.cores):
        xs = x[c * cfg.NP:(c + 1) * cfg.NP].astype(bf16)
        xTc = xT.copy()
        xTc[:F, ph] = xs.T
        dv = np.zeros(cfg.PL, dtype=np.float32)
        dv[ph] = dinv[c * cfg.NP:(c + 1) * cfg.NP]
        rdv = np.zeros(cfg.PL, dtype=np.float32)
        rdv[ph] = 1.0 / dv[ph]
        in_maps.append({
            "xT": xTc,
            "w1": W1p,
            "b1r": b1r,
            "w2": W2.astype(np.float32),
            "b2r": b2r,
            "ident": ident,
            "dinv": tileize(dv),
            "dinvh": tileize((1.0 - cfg.alpha) * dv),
            "dinv2h": tileize((1.0 - cfg.alpha) * dv * dv),
            "rdinv": tileize(rdv),
            "gidx": gidx_all[c],
            "lidx": lidx_all[c],
        })
    return in_maps, plan


# ------------------------------------------------------------- graph builder

def build_graph(cfg, plan, compile_for_hw=True):
    import concourse.bass as bass
    import concourse.bacc as bacc
    import concourse.mybir as mybir
    from concourse.library_config import mlp

    f32 = mybir.dt.float32
    bf16 = mybir.dt.bfloat16
    i16 = mybir.dt.int16
    H, O, T, PL, TH, HALF = cfg.H, cfg.O, cfg.T, cfg.PL, cfg.TH, cfg.HALF
    NS, NQ, LS, TG = cfg.NS, cfg.NQ, cfg.LS, cfg.TG
    S1, L2N = plan.S1, plan.L2N
    S16, L16 = S1 // 16, L2N // 16
    SP = cfg.SP

    # ---- call ordering: phase A (chunks 0,1) then phase B (chunks 2,3)
    per_chunk = [[b for b in plan.calls if b[0] == c] for c in range(cfg.NCH)]

    def interleave(a, b):
        out = []
        for i in range(max(len(a), len(b))):
            if i < len(a):
                out.append(a[i])
            if i < len(b):
                out.append(b[i])
        return out

    callsA = interleave(per_chunk[0], per_chunk[1])
    callsB = interleave(per_chunk[2], per_chunk[3])
    CALLS = callsA + callsB
    nA = len(callsA)
    NC = len(CALLS)

    # level-2 calls: all region A (14), then region B (14)
    l2A = [b for b in plan.l2 if b[0] == 0]
    l2B = [b for b in plan.l2 if b[0] == 1]
    L2C = l2A + l2B
    NL = len(L2C)

    # unified issue order (gpsimd + DVE): phase A mains, then phase B mains
    # with region-A L2 calls spliced in (their BARA input is ready ~20us
    # after phase A drains), then region-B L2 calls.
    def build_order(lead, every):
        order = [("m", k) for k in range(nA)]
        ai = 0
        for bpos, k in enumerate(range(nA, NC)):
            order.append(("m", k))
            if bpos >= lead and ai < len(l2A) and (bpos - lead) % every == 0:
                order.append(("l", ai))
                ai += 1
        while ai < len(l2A):
            order.append(("l", ai))
            ai += 1
        order += [("l", len(l2A) + j) for j in range(len(l2B))]
        return order

    LEAD, EVERY = 10, 4
    ISSUE = build_order(LEAD, EVERY)          # gpsimd issue order
    DVE_ISSUE = build_order(LEAD + 6, EVERY)  # DVE: tail trees before combines

    # -------- static schedule: absolute per-slot semaphore counts ----------
    gd_tot = [0] * NS
    dv_tot = [0] * NS
    pw_tot = [0] * NS
    lg_tot = [0] * (2 * LS)
    MAIN = []              # MAIN[t][k] = (b, s, gd_prior, dv_prior, pw_prior)
    BARA = []              # BARA[t] = (slot, pw_target) after last phase-A call
    BARB = []              # BARB[t] = (slot, pw_target) after last call
    L2S = []               # L2S[t][j] = (b, sl, lg_prior)
    for t in range(cfg.K):
        it = []
        for k, b in enumerate(CALLS):
            s = k % NS
            it.append((b, s, gd_tot[s], dv_tot[s], pw_tot[s]))
            gd_tot[s] += 1
            if b[3] >= 2:
                dv_tot[s] += 1
            pw_tot[s] += 1
            if k == nA - 1:
                BARA.append((s, pw_tot[s]))
        sB = it[-1][1]
        BARB.append((sB, pw_tot[sB]))
        MAIN.append(it)
        l2 = []
        for j, b in enumerate(L2C):
            r = b[0]
            sl = r * LS + (j % LS)
            l2.append((b, sl, lg_tot[sl]))
            lg_tot[sl] += 1
        L2S.append(l2)

    nc = bacc.Bacc("TRN2", target_bir_lowering=False, debug=False,
                   num_devices=cfg.cores, num_swdge_queues=NQ,
                   dynamic_dma_scratch_size=cfg.DMA_SCRATCH)

    xT_h = nc.declare_dram_parameter("xT", [cfg.FP, PL], bf16, isOutput=False)
    w1_h = nc.declare_dram_parameter("w1", [cfg.FP, H], bf16, isOutput=False)
    b1r_h = nc.declare_dram_parameter("b1r", [128, H], f32, isOutput=False)
    w2_h = nc.declare_dram_parameter("w2", [H, O], f32, isOutput=False)
    b2r_h = nc.declare_dram_parameter("b2r", [128, O], f32, isOutput=False)
    id_h = nc.declare_dram_parameter("ident", [128, 128], f32, isOutput=False)
    dinv_h = nc.declare_dram_parameter("dinv", [128, T], f32, isOutput=False)
    dinvh_h = nc.declare_dram_parameter("dinvh", [128, T], f32, isOutput=False)
    dinv2h_h = nc.declare_dram_parameter("dinv2h", [128, T], f32, isOutput=False)
    rdinv_h = nc.declare_dram_parameter("rdinv", [128, T], f32, isOutput=False)
    gidx_h = nc.declare_dram_parameter("gidx", [128, S16], i16, isOutput=False)
    lidx_h = nc.declare_dram_parameter("lidx", [128, L16], i16, isOutput=False)
    out_h = nc.declare_dram_parameter("out", [T, 128, O], f32, isOutput=True)

    cc0 = nc.dram_tensor("cc0", [HALF, H], f32)
    cc1 = nc.dram_tensor("cc1", [HALF, H], f32)
    gf0 = nc.dram_tensor("gf0", [cfg.cores * HALF, H], f32, addr_space="Shared")
    gf1 = nc.dram_tensor("gf1", [cfg.cores * HALF, H], f32, addr_space="Shared")
    partials = nc.dram_tensor("partials", [cfg.NCH * PL, H], f32)

    # lin1 m-groups
    MGW = []
    rem = PL
    while rem > 0:
        w = min(768, rem)
        MGW.append(w)
        rem -= w

    rg = [list(range(cfg.cores))]

    ctxs = []

    def sb(name, shape, dtype):
        cm = nc.sbuf_tensor(name, shape, dtype)
        h = cm.__enter__()
        ctxs.append(cm)
        return h

    def ps(name, shape, dtype):
        cm = nc.psum_tensor(name, shape, dtype)
        h = cm.__enter__()
        ctxs.append(cm)
        return h

    def sem(name):
        cm = nc.semaphore(name)
        h = cm.__enter__()
        ctxs.append(cm)
        return h

    SLOT_F32 = cfg.SLOTC * H                      # per-partition f32 per slot
    stage = sb("stage", [128, NS * SLOT_F32], f32)
    # level-2 staging; doubles as lin1 xT staging during setup (bf16 bitcast)
    l2st = sb("l2st", [128, 2 * LS * TG * 2 * H], f32)
    assert 2 * LS * TG * 2 * H >= 2 * 768 * cfg.KT // 2
    idxg_sb = sb("idxg", [128, S16], i16)
    idxl_sb = sb("idxl", [128, L16], i16)
    g0a_sb = sb("g0a", [128, T, H], f32)
    g_sb = sb("g", [128, T, H], f32)
    msum_sb = sb("msum_sb", [128, T, H], f32)
    w1_sb = sb("w1_sb", [128, cfg.KT, H], bf16)
    w2_sb = sb("w2_sb", [H, O], f32)
    b1r_sb = sb("b1r_sb", [128, H], f32)
    b2r_sb = sb("b2r_sb", [128, O], f32)
    id_sb = sb("id_sb", [128, 128], f32)
    dinv_sb = sb("dinv_sb", [128, T], f32)
    dinvh_sb = sb("dinvh_sb", [128, T], f32)
    dinv2h_sb = sb("dinv2h_sb", [128, T], f32)
    rdinv_sb = sb("rdinv_sb", [128, T], f32)
    tmp_sb = sb("tmp_sb", [128, 2, H], f32)
    EG = 7                                    # epilogue tile group
    NGE = T // EG                             # 14 groups
    ah_sb = sb("ah_sb", [128, EG, H], f32)
    u_sb = sb("u_sb", [128, 2, EG, H], f32)
    ur_sb = sb("ur_sb", [128, 2, EG, H], f32)
    lhsT_sb = sb("lhsT_sb", [H, EG, 128], f32)
    out_sb = sb("out_sb", [128, 2, EG, O], f32)

    ps_mm = [ps("ps_mm0", [128, H], f32), ps("ps_mm1", [128, H], f32)]
    ps_tr = [ps("ps_tr0", [H, EG, 128], f32), ps("ps_tr1", [H, EG, 128], f32)]
    ps_o = [ps("ps_o0", [128, EG, O], f32), ps("ps_o1", [128, EG, O], f32)]

    s_in = sem("s_in")
    s_x = [sem("s_x0"), sem("s_x1")]
    s_mm = sem("s_mm")
    s_ppf = sem("s_ppf")
    s_ep = sem("s_ep")
    s_gw = sem("s_gw")
    s_cc = sem("s_cc")
    s_gd = [sem(f"s_gd{i}") for i in range(NS)]   # gather landed (16/call)
    s_dv = [sem(f"s_dv{i}") for i in range(NS)]   # DVE tree done (1/call)
    s_pw = [sem(f"s_pw{i}") for i in range(NS)]   # partial written (16/call)
    s_lg = [sem(f"s_lg{i}") for i in range(2 * LS)]  # L2 landed
    s_lc = [sem(f"s_lc{i}") for i in range(2 * LS)]  # L2 consumed
    s_gu = sem("s_gu")
    s_prep = sem("s_prep")
    s_ah = sem("s_ah")
    s_u = sem("s_u")
    s_ur = sem("s_ur")
    s_tr = sem("s_tr")
    s_trc = sem("s_trc")
    s_mo = sem("s_mo")
    s_ob = sem("s_ob")
    s_ow = [sem("s_ow0"), sem("s_ow1")]

    def slot_view(s, nt, D):                  # [128, nt, D, H]
        base = s * SLOT_F32
        return stage[:, base:base + nt * D * H].rearrange(
            "p (n d m) -> p n d m", d=D, m=H)

    def slot_flat(s, ncols):                  # [128, ncols, H] gather target
        base = s * SLOT_F32
        return stage[:, base:base + ncols * H].rearrange(
            "p (n m) -> p n m", m=H)

    def l2_view(r, ls, nt):                   # [128, nt, 2, H]
        base = (r * LS + ls) * TG * 2 * H
        return l2st[:, base:base + nt * 2 * H].rearrange(
            "p (n k m) -> p n k m", k=2, m=H)

    def stage_slot_x(s, kt, w):               # [128, kt, w] bf16 xT view (l2st)
        wlen = 768 * kt // 2                  # f32 words per slot (bf16 data)
        return l2st[:, s * wlen:(s + 1) * wlen].bitcast(bf16).rearrange(
            "p (k m) -> p k m", k=kt)[:, :, :w]

    def gsrc(c):                              # gather source slice for chunk c
        gf = gf0 if c < 2 else gf1
        q = c % 2
        return gf[q * cfg.CH:(q + 1) * cfg.CH, :]

    xT3 = xT_h.ap().rearrange("(k p) m -> p k m", p=128)
    w13 = w1_h.ap().rearrange("(k p) m -> p k m", p=128)
    cc0_3 = cc0.ap().rearrange("(n p) m -> p n m", p=128)
    cc1_3 = cc1.ap().rearrange("(n p) m -> p n m", p=128)

    n_in_dmas = 12

    tiles_per_group = [(w + 127) // 128 for w in MGW]
    cum_tiles = np.cumsum([0] + tiles_per_group)

    # ============================== setup block ==============================
    with nc.Block() as blk:
        @blk.sync
        def _(sy):
            sy.dma_start(idxg_sb[:, :], gidx_h[:, :]).then_inc(s_in, 16)
            sy.dma_start(idxl_sb[:, :], lidx_h[:, :]).then_inc(s_in, 16)
            sy.dma_start(w1_sb[:, :, :], w13).then_inc(s_in, 16)
            sy.dma_start(w2_sb[:, :], w2_h[:, :]).then_inc(s_in, 16)
            sy.dma_start(b1r_sb[:, :], b1r_h[:, :]).then_inc(s_in, 16)
            sy.dma_start(b2r_sb[:, :], b2r_h[:, :]).then_inc(s_in, 16)
            sy.dma_start(id_sb[:, :], id_h[:, :]).then_inc(s_in, 16)
            sy.dma_start(dinv_sb[:, :], dinv_h[:, :]).then_inc(s_in, 16)
            sy.dma_start(dinvh_sb[:, :], dinvh_h[:, :]).then_inc(s_in, 16)
            sy.dma_start(dinv2h_sb[:, :], dinv2h_h[:, :]).then_inc(s_in, 16)
            sy.dma_start(rdinv_sb[:, :], rdinv_h[:, :]).then_inc(s_in, 16)
            sy.wait_ge(s_in, 16 * (n_in_dmas - 1))
            col = 0
            for mg, w in enumerate(MGW):
                if mg >= 2:
                    sy.wait_ge(s_mm, int(cum_tiles[mg - 1]))
                sy.dma_start(stage_slot_x(mg % 2, cfg.KT, w),
                             xT3[:, :, col:col + w]).then_inc(s_x[mg % 2], 16)
                col += w
            sy.wait_ge(s_ep, TH)
            sy.dma_start(cc0_3, g_sb[:, 0:TH, :]).then_inc(s_gw, 16)
            sy.wait_ge(s_ep, T)
            sy.dma_start(cc1_3, g_sb[:, TH:T, :]).then_inc(s_gw, 16)
            sy.wait_ge(s_gw, 32)

        @blk.tensor
        def _(pe):
            pe.wait_ge(s_in, 16 * (n_in_dmas - 1))
            ti = 0
            for mg, w in enumerate(MGW):
                pe.wait_ge(s_x[mg % 2], 16 * (mg // 2 + 1))
                nt = tiles_per_group[mg]
                for m in range(nt):
                    mw = min(128, w - m * 128)
                    if ti >= 2:
                        pe.wait_ge(s_ppf, ti - 1)
                    for k in range(cfg.KT):
                        ins = pe.matmul(
                            ps_mm[ti % 2][:mw, :],
                            stage_slot_x(mg % 2, cfg.KT, w)[:, k, m * 128:m * 128 + mw],
                            w1_sb[:, k, :],
                            start=(k == 0), stop=(k == cfg.KT - 1))
                        if k == cfg.KT - 1:
                            ins.then_inc(s_mm)
                    ti += 1

        @blk.vector
        def _(ve):
            ve.wait_ge(s_in, 16 * (n_in_dmas - 1))
            for ti in range(T):
                ve.wait_ge(s_mm, ti + 1)
                if ti >= 2:
                    ve.wait_ge(s_ep, ti - 1)   # Act consumed tmp slot
                ve.tensor_add(tmp_sb[:, ti % 2, :], ps_mm[ti % 2][:, :],
                              b1r_sb[:, :]).then_inc(s_ppf)

        @blk.scalar
        def _(ac):
            ac.wait_ge(s_in, 16 * (n_in_dmas - 1))
            for ti in range(T):
                ac.wait_ge(s_ppf, ti + 1)
                ac.activation(g0a_sb[:, ti, :], tmp_sb[:, ti % 2, :],
                              mybir.ActivationFunctionType.Copy,
                              scale=dinvh_sb[:, ti:ti + 1])
                ac.activation(g_sb[:, ti, :], tmp_sb[:, ti % 2, :],
                              mybir.ActivationFunctionType.Copy,
                              scale=dinv_sb[:, ti:ti + 1]).then_inc(s_ep)

        @blk.gpsimd
        def _(gp):
            gp.load_library(mlp)
            gp.wait_ge(s_gw, 16)
            gp.collective_compute(
                "AllGather", mybir.AluOpType.bypass, replica_groups=rg,
                ins=[cc0.ap().opt()], outs=[gf0.ap().opt()],
            ).then_inc(s_cc)
            gp.wait_ge(s_gw, 32)
            gp.collective_compute(
                "AllGather", mybir.AluOpType.bypass, replica_groups=rg,
                ins=[cc1.ap().opt()], outs=[gf1.ap().opt()],
            ).then_inc(s_cc)

    # ============================ iteration blocks ===========================
    for t in range(cfg.K):
        last = (t == cfg.K - 1)
        with nc.Block() as blk:
            @blk.gpsimd
            def _(gp, t=t, last=last):
                gp.wait_ge(s_cc, 2 * t + 1)
                bar_done = [False, False]
                for kind, k in ISSUE:
                    if kind == "m":
                        (c, coff, ncols, D, nt, t0), s, gdp, dvp, pwp = MAIN[t][k]
                        if k == nA:
                            gp.wait_ge(s_cc, 2 * t + 2)
                        if pwp > 0:
                            gp.wait_ge(s_pw[s], 16 * pwp)
                        nb = ncols * 128
                        off = coff * 128
                        gp.dma_gather(
                            slot_flat(s, ncols),
                            gsrc(c),
                            idxg_sb[:, off // 16:(off + nb) // 16],
                            nb, nb, H, elem_step=H, queue_num=k % NQ,
                            single_packet=SP,
                        ).then_inc(s_gd[s], 16)
                    else:
                        (r, t0, nt, loff), sl, lgp = L2S[t][k]
                        if not bar_done[r]:
                            bs, btgt = (BARA[t] if r == 0 else BARB[t])
                            gp.wait_ge(s_pw[bs], 16 * btgt)
                            bar_done[r] = True
                        if lgp > 0:
                            gp.wait_ge(s_lc[sl], lgp)
                        nb = nt * 2 * 128
                        gp.dma_gather(
                            l2_view(r, sl - r * LS, nt).rearrange(
                                "p n k m -> p (n k) m"),
                            partials[2 * r * PL:(2 * r + 2) * PL, :],
                            idxl_sb[:, loff // 16:(loff + nb) // 16],
                            nb, nb, H, elem_step=H, queue_num=k % NQ,
                            single_packet=SP,
                        ).then_inc(s_lg[sl], 16)
                if not last:
                    gp.wait_ge(s_gw, 16 * (2 * t + 3))
                    gp.collective_compute(
                        "AllGather", mybir.AluOpType.bypass, replica_groups=rg,
                        ins=[cc0.ap().opt()], outs=[gf0.ap().opt()],
                    ).then_inc(s_cc)
                    gp.wait_ge(s_gw, 16 * (2 * t + 4))
                    gp.collective_compute(
                        "AllGather", mybir.AluOpType.bypass, replica_groups=rg,
                        ins=[cc1.ap().opt()], outs=[gf1.ap().opt()],
                    ).then_inc(s_cc)

            @blk.vector
            def _(ve, t=t, last=last):
                # trees + level-2 combines; combines offset later than issue
                for kind, k in DVE_ISSUE:
                    if kind == "m":
                        (c, coff, ncols, D, nt, t0), s, gdp, dvp, pwp = MAIN[t][k]
                        ve.wait_ge(s_gd[s], 16 * (gdp + 1))
                        if D >= 2:
                            v = slot_view(s, nt, D)
                            cur = D
                            ins = None
                            while cur > 1:
                                h = cur // 2
                                lo = cur - h
                                ins = ve.tensor_add(v[:, :, 0:h, :],
                                                    v[:, :, 0:h, :],
                                                    v[:, :, lo:lo + h, :])
                                cur = lo
                            ins.then_inc(s_dv[s])
                        continue
                    (r, t0, nt, loff), sl, lgp = L2S[t][k]
                    ve.wait_ge(s_lg[sl], 16 * (lgp + 1))
                    v = l2_view(r, sl - r * LS, nt)
                    mr = msum_sb[:, t0:t0 + nt, :]
                    if r == 0:
                        ve.tensor_add(mr, v[:, :, 0, :],
                                      v[:, :, 1, :]).then_inc(s_lc[sl])
                    else:
                        ve.tensor_add(mr, mr, v[:, :, 0, :])
                        ve.tensor_add(mr, mr, v[:, :, 1, :]).then_inc(s_lc[sl])
                        ins = ve.tensor_add(mr, mr, g_sb[:, t0:t0 + nt, :])
                        if not last:
                            for ti in range(t0, t0 + nt):
                                ins = ve.scalar_tensor_tensor(
                                    g_sb[:, ti, :], msum_sb[:, ti, :],
                                    dinv2h_sb[:, ti:ti + 1], g0a_sb[:, ti, :],
                                    mybir.AluOpType.mult, mybir.AluOpType.add)
                                if ti == TH - 1 or ti == T - 1:
                                    ins.then_inc(s_gu)
                        else:
                            if t0 + nt == TH or t0 + nt == T:
                                ins.then_inc(s_gu)

            @blk.sync
            def _(sy, t=t, last=last):
                # protect previous iteration's L2 reads of `partials`
                if t >= 1:
                    sy.wait_ge(s_gu, 2 * t)
                # partial write-outs (issue order; HWDGE FIFO per engine)
                for (c, coff, ncols, D, nt, t0), s, gdp, dvp, pwp in MAIN[t]:
                    if D >= 2:
                        sy.wait_ge(s_dv[s], dvp + 1)
                    else:
                        sy.wait_ge(s_gd[s], 16 * (gdp + 1))
                    v = slot_view(s, nt, D)
                    dst = partials[c * PL + t0 * 128:
                                   c * PL + (t0 + nt) * 128, :].rearrange(
                                       "(n p) m -> p n m", p=128)
                    sy.dma_start(dst, v[:, :, 0, :]).then_inc(s_pw[s], 16)
                if not last:
                    sy.wait_ge(s_gu, 2 * t + 1)
                    sy.dma_start(cc0_3, g_sb[:, 0:TH, :]).then_inc(s_gw, 16)
                    sy.wait_ge(s_gu, 2 * t + 2)
                    sy.dma_start(cc1_3, g_sb[:, TH:T, :]).then_inc(s_gw, 16)
                    sy.wait_ge(s_gw, 16 * (2 * t + 4))

    # ============================== epilogue =================================
    # batched by EG-tile groups: per-engine ops run back-to-back inside a
    # group, semaphores hop once per group instead of once per tile.
    with nc.Block() as blk:
        @blk.scalar
        def _(ac):
            for g in range(NGE):
                sl = g % 2
                if g >= 1:
                    ac.wait_ge(s_u, g)          # DVE consumed ah (single buf)
                for i in range(EG):
                    ti = g * EG + i
                    ins = ac.activation(ah_sb[:, i, :], g0a_sb[:, ti, :],
                                        mybir.ActivationFunctionType.Copy,
                                        scale=rdinv_sb[:, ti:ti + 1])
                ins.then_inc(s_ah)
                ac.wait_ge(s_u, g + 1)
                if g >= 2:
                    ac.wait_ge(s_tr, g - 1)     # PE consumed ur slot
                ac.activation(
                    ur_sb[:, sl, :, :].rearrange("p n m -> p (n m)"),
                    u_sb[:, sl, :, :].rearrange("p n m -> p (n m)"),
                    mybir.ActivationFunctionType.Relu).then_inc(s_ur)

        @blk.vector
        def _(ve):
            ve.wait_ge(s_gu, 2 * cfg.K - 1)
            for g in range(NGE):
                sl = g % 2
                if g * EG == TH:
                    ve.wait_ge(s_gu, 2 * cfg.K)
                ve.wait_ge(s_ah, g + 1)
                if g >= 2:
                    ve.wait_ge(s_ur, g - 1)     # Act consumed u slot
                for i in range(EG):
                    ti = g * EG + i
                    ins = ve.scalar_tensor_tensor(
                        u_sb[:, sl, i, :], msum_sb[:, ti, :],
                        dinvh_sb[:, ti:ti + 1], ah_sb[:, i, :],
                        mybir.AluOpType.mult, mybir.AluOpType.add)
                ins.then_inc(s_u)
                ve.wait_ge(s_tr, g + 1)
                if g >= 1:
                    ve.wait_ge(s_mo, g)         # PE consumed lhsT (single buf)
                ve.tensor_copy(
                    lhsT_sb[:, :, :].rearrange("p n m -> p (n m)"),
                    ps_tr[sl][:, :, :].rearrange("p n m -> p (n m)"),
                ).then_inc(s_trc)
                ve.wait_ge(s_mo, g + 1)
                if g >= 2:
                    ve.wait_ge(s_ow[sl], 16 * (g // 2))
                for i in range(EG):
                    ins = ve.tensor_add(out_sb[:, sl, i, :],
                                        ps_o[sl][:, i, :], b2r_sb[:, :])
                ins.then_inc(s_ob)

        @blk.tensor
        def _(pe):
            for g in range(NGE):
                sl = g % 2
                pe.wait_ge(s_ur, g + 1)
                if g >= 2:
                    pe.wait_ge(s_trc, g - 1)    # DVE consumed ps_tr slot
                for i in range(EG):
                    ins = pe.transpose(ps_tr[sl][:, i, :], ur_sb[:, sl, i, :],
                                       id_sb[:, :])
                ins.then_inc(s_tr)
                pe.wait_ge(s_trc, g + 1)
                if g >= 2:
                    pe.wait_ge(s_ob, g - 1)     # DVE consumed ps_o slot
                for i in range(EG):
                    ins = pe.matmul(ps_o[sl][:, i, :], lhsT_sb[:, i, :],
                                    w2_sb[:, :], start=True, stop=True)
                ins.then_inc(s_mo)

        @blk.sync
        def _(sy):
            for g in range(NGE):
                sy.wait_ge(s_ob, g + 1)
                dst = out_h[g * EG:(g + 1) * EG, :, :].rearrange("n p m -> p n m")
                sy.dma_start(dst, out_sb[:, g % 2, :, :]).then_inc(
                    s_ow[g % 2], 16)
            for par in range(2):
                n_par = (NGE + 1 - par) // 2
                if n_par:
                    sy.wait_ge(s_ow[par], 16 * n_par)

    print(f"SBUF used: {(nc.sbuf_base + (nc.SBUF_PARTITION_SIZE_BYTES - nc.sbuf_top)) / 1024:.0f} KB/part "
          f"(base {nc.sbuf_base//1024}KB top-res {(nc.SBUF_PARTITION_SIZE_BYTES - nc.sbuf_top)//1024}KB of {nc.SBUF_PARTITION_SIZE_BYTES//1024}KB) "
          f"NC={NC} NL={NL} S1={S1} pad={plan.pad_frac:.3f} maxD={plan.maxD}")
    if compile_for_hw:
        nc.compile()
    return nc


# ----------------------------------------------------------------- kernel()

_CACHE = {}


def _run(cfg, inputs, trace=False):
    from concourse.bass_utils import run_bass_kernel_spmd

    in_maps, plan = host_prep(cfg, inputs["x"], inputs["edge_index"],
                              inputs["W1"], inputs["b1"],
                              inputs["W2"], inputs["b2"])
    key = (cfg.N, cfg.E, plan.S1, tuple(b[:4] for b in plan.calls))
    if key not in _CACHE:
        _CACHE[key] = build_graph(cfg, plan)
    nc = _CACHE[key]
    res = run_bass_kernel_spmd(nc, in_maps, list(range(cfg.cores)), trace=trace)
    ph = _phys_map(cfg)
    outs = []
    for c in range(cfg.cores):
        o = np.asarray(res.results[c]["out"]).reshape(cfg.PL, cfg.O)
        outs.append(o[ph])
    return np.concatenate(outs, axis=0), res


def kernel(**inputs):
    out, _ = _run(FULL, inputs)
    return out


# revision 25
# speedup vs baseline: 1.1386x; 1.1386x over previous
"""APPNP GNN kernel for 8 TRN2 NeuronCores — gather + on-chip reduce, v3.

Reference computation (N=100000 nodes, E=1600000 edges, K=5, alpha=0.5):
    h0 = x @ W1 + b1
    deg[d] = |in-edges(d)| + 1 (self loop); dinv = rsqrt(deg)
    5x: h = (1-a) * dinv * S(dinv * h) + a * h0     (S = adjacency sum + self)
    out = relu(h) @ W2 + b2

v3 structure (per core, nodes row-sharded 12500/core padded to 12544):
  Node rows are remapped to a physical layout with a zero row at 6271 and
  12501.. so each HALF (rows [0,6272) / [6272,12544)) contains a zero row.
  g_full is split as TWO shared tensors gf0/gf1 (one per source half), each
  all-gathered separately so the half-collectives pipeline under compute:
    chunk = 2*half + quad   (quad = src cores 0-3 / 4-7), 25088 rows each.
  Iteration: phase A gathers (chunks 0,1; needs only gf0) -> phase B
  (chunks 2,3; needs gf1) -> partial write-outs -> L2 unsort-gathers
  (region A = chunks {0,1}, B = {2,3}) -> DVE combine + g update per
  7-tile group; when tiles 0-48 are updated, cc0 is written and gf0's
  AllGather for the next iteration fires (while tiles 49-97 still combine),
  then cc1/gf1.  Desc-gen on the Pool engine (~2.3ns/desc serialized) is
  the bottleneck; everything else hides behind it.
  lin1 runs in bf16 (xT + W1 staged bf16).  Epilogue is batched by 7-tile
  groups (one semaphore hop per group, ops back-to-back per engine):
  u = relu(dinvh*msum + a*h0); out = u @ W2 + b2 (PE transpose).
  10 gather slots (k%10), queue k%4, 48KB SWDGE ring; region-A L2 gathers
  are spliced into phase B so only region B sits on the iteration tail.
  lin1 xT streams in 768-col bf16 groups.
  Measured: 4.313ms HW (baseline 4.652ms), rel err 2.6e-3.
"""

import math
import numpy as np

# ----------------------------------------------------------------- config

class Cfg:
    def __init__(self, N=100000, E=1600000, F=500, H=64, O=40, K=5, alpha=0.5,
                 cores=8, batch_slots=4096):
        self.N, self.E, self.F, self.H, self.O, self.K = N, E, F, H, O, K
        self.alpha = alpha
        self.cores = cores
        assert N % cores == 0
        self.NP = N // cores                      # real nodes per core
        self.PL = ((self.NP + 127) // 128) * 128  # padded nodes per core
        self.T = self.PL // 128                   # tiles per core
        self.HALF = self.PL // 2                  # rows per half (6272)
        self.TH = self.T // 2                     # tiles per half (49)
        self.FP = ((F + 127) // 128) * 128        # padded feature dim
        self.KT = self.FP // 128                  # k tiles for lin1
        self.NCH = 4                              # chunks: (half, quad)
        self.CH = cores // 2 * self.HALF          # chunk rows (25088)
        assert self.CH <= 32600
        self.DMA_SCRATCH = 49152                  # SWDGE ring carveout bytes
        self.NQ = 4                               # SWDGE queues
        self.NS = 10                              # gather slots (k % NS)
        self.SLOTC = 15                           # columns per gather slot
        self.BS = batch_slots                     # max slots per gather call
        self.TG = 7                               # level-2 tile group (49%7==0)
        self.LS = 2                               # level-2 slots per region
        self.SP = False                           # single_packet for gathers

FULL = Cfg()


def _phys_map(cfg):
    """real local index (0..NP-1) -> physical row (zero rows at HALF-1 and
    NP+1..PL-1 physical positions)."""
    a = np.arange(cfg.NP, dtype=np.int64)
    return np.where(a < cfg.HALF - 1, a, a + 1)

# ----------------------------------------------------------- host preprocess

def _wrap16(arr):
    w = arr.reshape(-1, 16).T.astype(np.int16)       # [16, S/16]
    return np.ascontiguousarray(np.tile(w, (8, 1)))  # [128, S/16]


def build_plan(cfg, edge_index):
    """Shared (core-independent) call structure + per-core index tables."""
    N, PL, NP, CH, HALF = cfg.N, cfg.PL, cfg.NP, cfg.CH, cfg.HALF
    src = np.asarray(edge_index[0], dtype=np.int64)
    dst = np.asarray(edge_index[1], dtype=np.int64)
    ph = _phys_map(cfg)

    core_of_dst = dst // NP
    dst_loc = ph[dst % NP]                        # physical row of dst
    core_s = src // NP
    loc_s = ph[src % NP]                          # physical row of src
    half_s = loc_s // HALF
    chunk = 2 * half_s + core_s // (cfg.cores // 2)
    src_loc = (core_s % (cfg.cores // 2)) * HALF + (loc_s % HALF)

    # per (core, chunk, dst) degrees
    key = (core_of_dst * cfg.NCH + chunk) * PL + dst_loc
    counts = np.bincount(key, minlength=cfg.cores * cfg.NCH * PL)
    d = counts.reshape(cfg.cores, cfg.NCH, PL)

    # degree-descending order per (core, chunk); pos = rank of node
    ordr = np.argsort(-d, axis=2, kind="stable")           # [co, c, PL]
    pos = np.empty_like(ordr)
    ar = np.arange(PL)
    for co in range(cfg.cores):
        for c in range(cfg.NCH):
            pos[co, c, ordr[co, c]] = ar

    # shared per-tile degree: max over cores of tile-max (desc sort -> first)
    dsort = np.take_along_axis(d, ordr, axis=2)            # [co, c, PL] desc
    Dtile = dsort[:, :, ::128].max(axis=0)                 # [c, T]
    Dtile = np.maximum(Dtile, 1)
    maxD = int(Dtile.max())
    assert maxD <= cfg.SLOTC, f"tile degree {maxD} exceeds slot cols"

    # main gather calls: runs of equal-D tiles, <= SLOTC cols
    calls = []           # (chunk, col_off, ncols, D, nt, t0)
    colbase = np.zeros((cfg.NCH, cfg.T), dtype=np.int64)
    col = 0
    for c in range(cfg.NCH):
        t = 0
        while t < cfg.T:
            D = int(Dtile[c, t])
            nt = 1
            while (t + nt < cfg.T and int(Dtile[c, t + nt]) == D
                   and (nt + 1) * D <= cfg.SLOTC
                   and (nt + 1) * D * 128 <= cfg.BS):
                nt += 1
            colbase[c, t:t + nt] = col + np.arange(nt) * D
            calls.append((c, col, nt * D, D, nt, t))
            col += nt * D
            t += nt
    Ctot = col
    S1 = 128 * Ctot

    # per-core main index tables
    o1 = np.argsort(key, kind="stable")
    ks = key[o1]
    first = np.r_[True, ks[1:] != ks[:-1]]
    starts = np.where(first, np.arange(len(ks)), 0)
    occ = np.arange(len(ks)) - np.maximum.accumulate(starts)
    occ_e = np.empty_like(occ)
    occ_e[o1] = occ

    ZR = HALF - 1                                 # zero row in every chunk
    gidx_all = []
    rank_e = pos[core_of_dst, chunk, dst_loc]     # rank of dst in (co,chunk)
    t_e = rank_e // 128
    p_e = rank_e % 128
    slot_e = (colbase[chunk, t_e] + occ_e) * 128 + p_e
    for co in range(cfg.cores):
        flat = np.full(S1, ZR, dtype=np.int16)
        m = core_of_dst == co
        flat[slot_e[m]] = src_loc[m].astype(np.int16)
        gidx_all.append(_wrap16(flat))

    # level-2: region r, natural tile t, k in {0,1} -> chunk 2r+k
    L2N = 2 * cfg.T * 2 * 128                     # slots over both regions
    l2_calls = []
    for r in range(2):
        for t0 in range(0, cfg.T, cfg.TG):
            nt = min(cfg.TG, cfg.T - t0)
            l2_calls.append((r, t0, nt, r * L2N // 2 + t0 * 2 * 128))
    lidx_all = []
    nodes = np.arange(PL)
    for co in range(cfg.cores):
        flat = np.empty(L2N, dtype=np.int16)
        for r in range(2):
            for k in range(2):
                v = (k * PL + pos[co, 2 * r + k, nodes]).astype(np.int16)
                sl = ((nodes // 128) * 2 + k) * 128 + nodes % 128
                flat[r * (L2N // 2) + sl] = v
        lidx_all.append(_wrap16(flat))

    class Plan:
        pass
    plan = Plan()
    plan.calls = calls
    plan.l2 = l2_calls
    plan.S1 = S1
    plan.L2N = L2N
    plan.maxD = maxD
    plan.pad_frac = (S1 - int(np.sum(d)) / cfg.cores) / max(S1, 1)
    return plan, gidx_all, lidx_all


def host_prep(cfg, x, edge_index, W1, b1, W2, b2):
    import ml_dtypes
    bf16 = ml_dtypes.bfloat16
    N, H, F = cfg.N, cfg.H, cfg.F
    dst = np.asarray(edge_index[1], dtype=np.int64)
    deg = np.bincount(dst, minlength=N).astype(np.float64) + 1.0
    dinv = (1.0 / np.sqrt(deg)).astype(np.float32)
    ph = _phys_map(cfg)

    plan, gidx_all, lidx_all = build_plan(cfg, edge_index)

    def tileize(v):  # [PL] -> [128, T]
        return np.ascontiguousarray(v.reshape(cfg.T, 128).T)

    xT = np.zeros((cfg.FP, cfg.PL), dtype=bf16)
    W1p = np.zeros((cfg.FP, H), dtype=bf16)
    W1p[:F] = W1.astype(bf16)
    b1r = np.ascontiguousarray(np.broadcast_to(b1.astype(np.float32), (128, H)))
    b2r = np.ascontiguousarray(np.broadcast_to(b2.astype(np.float32), (128, cfg.O)))
    ident = np.eye(128, dtype=np.float32)

    in_maps = []
    for c in range(cfg.cores):
        xs = x[c * cfg.NP:(c + 1) * cfg.NP].astype(bf16)
        xTc = xT.copy()
        xTc[:F, ph] = xs.T
        dv = np.zeros(cfg.PL, dtype=np.float32)
        dv[ph] = dinv[c * cfg.NP:(c + 1) * cfg.NP]
        rdv = np.zeros(cfg.PL, dtype=np.float32)
        rdv[ph] = 1.0 / dv[ph]
        in_maps.append({
            "xT": xTc,
            "w1": W1p,
            "b1r": b1r,
            "w2": W2.astype(np.float32),
            "b2r": b2r,
            "ident": ident,
            "dinv": tileize(dv),
            "dinvh": tileize((1.0 - cfg.alpha) * dv),
            "dinv2h": tileize((1.0 - cfg.alpha) * dv * dv),
            "rdinv": tileize(rdv),
            "gidx": gidx_all[c],
            "lidx": lidx_all[c],
        })
    return in_maps, plan


# ------------------------------------------------------------- graph builder

def build_graph(cfg, plan, compile_for_hw=True):
    import concourse.bass as bass
    import concourse.bacc as bacc
    import concourse.mybir as mybir
    from concourse.library_config import mlp

    f32 = mybir.dt.float32
    bf16 = mybir.dt.bfloat16
    i16 = mybir.dt.int16
    H, O, T, PL, TH, HALF = cfg.H, cfg.O, cfg.T, cfg.PL, cfg.TH, cfg.HALF
    NS, NQ, LS, TG = cfg.NS, cfg.NQ, cfg.LS, cfg.TG
    S1, L2N = plan.S1, plan.L2N
    S16, L16 = S1 // 16, L2N // 16
    SP = cfg.SP

    # ---- call ordering: phase A (chunks 0,1) then phase B (chunks 2,3)
    per_chunk = [[b for b in plan.calls if b[0] == c] for c in range(cfg.NCH)]

    def interleave(a, b):
        out = []
        for i in range(max(len(a), len(b))):
            if i < len(a):
                out.append(a[i])
            if i < len(b):
                out.append(b[i])
        return out

    callsA = interleave(per_chunk[0], per_chunk[1])
    callsB = interleave(per_chunk[2], per_chunk[3])
    CALLS = callsA + callsB
    nA = len(callsA)
    NC = len(CALLS)

    # level-2 calls: all region A (14), then region B (14)
    l2A = [b for b in plan.l2 if b[0] == 0]
    l2B = [b for b in plan.l2 if b[0] == 1]
    L2C = l2A + l2B
    NL = len(L2C)

    # unified issue order (gpsimd + DVE): phase A mains, then phase B mains
    # with region-A L2 calls spliced in (their BARA input is ready ~20us
    # after phase A drains), then region-B L2 calls.
    ISSUE = [("m", k) for k in range(nA)]
    LEAD, EVERY = 14, 4
    ai = 0
    for bpos, k in enumerate(range(nA, NC)):
        ISSUE.append(("m", k))
        if bpos >= LEAD and ai < len(l2A) and (bpos - LEAD) % EVERY == 0:
            ISSUE.append(("l", ai))
            ai += 1
    while ai < len(l2A):
        ISSUE.append(("l", ai))
        ai += 1
    ISSUE += [("l", len(l2A) + j) for j in range(len(l2B))]

    # -------- static schedule: absolute per-slot semaphore counts ----------
    gd_tot = [0] * NS
    dv_tot = [0] * NS
    pw_tot = [0] * NS
    lg_tot = [0] * (2 * LS)
    MAIN = []              # MAIN[t][k] = (b, s, gd_prior, dv_prior, pw_prior)
    BARA = []              # BARA[t] = (slot, pw_target) after last phase-A call
    BARB = []              # BARB[t] = (slot, pw_target) after last call
    L2S = []               # L2S[t][j] = (b, sl, lg_prior)
    for t in range(cfg.K):
        it = []
        for k, b in enumerate(CALLS):
            s = k % NS
            it.append((b, s, gd_tot[s], dv_tot[s], pw_tot[s]))
            gd_tot[s] += 1
            if b[3] >= 2:
                dv_tot[s] += 1
            pw_tot[s] += 1
            if k == nA - 1:
                BARA.append((s, pw_tot[s]))
        sB = it[-1][1]
        BARB.append((sB, pw_tot[sB]))
        MAIN.append(it)
        l2 = []
        for j, b in enumerate(L2C):
            r = b[0]
            sl = r * LS + (j % LS)
            l2.append((b, sl, lg_tot[sl]))
            lg_tot[sl] += 1
        L2S.append(l2)

    nc = bacc.Bacc("TRN2", target_bir_lowering=False, debug=False,
                   num_devices=cfg.cores, num_swdge_queues=NQ,
                   dynamic_dma_scratch_size=cfg.DMA_SCRATCH)

    xT_h = nc.declare_dram_parameter("xT", [cfg.FP, PL], bf16, isOutput=False)
    w1_h = nc.declare_dram_parameter("w1", [cfg.FP, H], bf16, isOutput=False)
    b1r_h = nc.declare_dram_parameter("b1r", [128, H], f32, isOutput=False)
    w2_h = nc.declare_dram_parameter("w2", [H, O], f32, isOutput=False)
    b2r_h = nc.declare_dram_parameter("b2r", [128, O], f32, isOutput=False)
    id_h = nc.declare_dram_parameter("ident", [128, 128], f32, isOutput=False)
    dinv_h = nc.declare_dram_parameter("dinv", [128, T], f32, isOutput=False)
    dinvh_h = nc.declare_dram_parameter("dinvh", [128, T], f32, isOutput=False)
    dinv2h_h = nc.declare_dram_parameter("dinv2h", [128, T], f32, isOutput=False)
    rdinv_h = nc.declare_dram_parameter("rdinv", [128, T], f32, isOutput=False)
    gidx_h = nc.declare_dram_parameter("gidx", [128, S16], i16, isOutput=False)
    lidx_h = nc.declare_dram_parameter("lidx", [128, L16], i16, isOutput=False)
    out_h = nc.declare_dram_parameter("out", [T, 128, O], f32, isOutput=True)

    cc0 = nc.dram_tensor("cc0", [HALF, H], f32)
    cc1 = nc.dram_tensor("cc1", [HALF, H], f32)
    gf0 = nc.dram_tensor("gf0", [cfg.cores * HALF, H], f32, addr_space="Shared")
    gf1 = nc.dram_tensor("gf1", [cfg.cores * HALF, H], f32, addr_space="Shared")
    partials = nc.dram_tensor("partials", [cfg.NCH * PL, H], f32)

    # lin1 m-groups
    MGW = []
    rem = PL
    while rem > 0:
        w = min(768, rem)
        MGW.append(w)
        rem -= w

    rg = [list(range(cfg.cores))]

    ctxs = []

    def sb(name, shape, dtype):
        cm = nc.sbuf_tensor(name, shape, dtype)
        h = cm.__enter__()
        ctxs.append(cm)
        return h

    def ps(name, shape, dtype):
        cm = nc.psum_tensor(name, shape, dtype)
        h = cm.__enter__()
        ctxs.append(cm)
        return h

    def sem(name):
        cm = nc.semaphore(name)
        h = cm.__enter__()
        ctxs.append(cm)
        return h

    SLOT_F32 = cfg.SLOTC * H                      # per-partition f32 per slot
    stage = sb("stage", [128, NS * SLOT_F32], f32)
    # level-2 staging; doubles as lin1 xT staging during setup (bf16 bitcast)
    l2st = sb("l2st", [128, 2 * LS * TG * 2 * H], f32)
    assert 2 * LS * TG * 2 * H >= 2 * 768 * cfg.KT // 2
    idxg_sb = sb("idxg", [128, S16], i16)
    idxl_sb = sb("idxl", [128, L16], i16)
    g0a_sb = sb("g0a", [128, T, H], f32)
    g_sb = sb("g", [128, T, H], f32)
    msum_sb = sb("msum_sb", [128, T, H], f32)
    w1_sb = sb("w1_sb", [128, cfg.KT, H], bf16)
    w2_sb = sb("w2_sb", [H, O], f32)
    b1r_sb = sb("b1r_sb", [128, H], f32)
    b2r_sb = sb("b2r_sb", [128, O], f32)
    id_sb = sb("id_sb", [128, 128], f32)
    dinv_sb = sb("dinv_sb", [128, T], f32)
    dinvh_sb = sb("dinvh_sb", [128, T], f32)
    dinv2h_sb = sb("dinv2h_sb", [128, T], f32)
    rdinv_sb = sb("rdinv_sb", [128, T], f32)
    tmp_sb = sb("tmp_sb", [128, 2, H], f32)
    EG = 7                                    # epilogue tile group
    NGE = T // EG                             # 14 groups
    ah_sb = sb("ah_sb", [128, EG, H], f32)
    u_sb = sb("u_sb", [128, 2, EG, H], f32)
    ur_sb = sb("ur_sb", [128, 2, EG, H], f32)
    lhsT_sb = sb("lhsT_sb", [H, EG, 128], f32)
    out_sb = sb("out_sb", [128, 2, EG, O], f32)

    ps_mm = [ps("ps_mm0", [128, H], f32), ps("ps_mm1", [128, H], f32)]
    ps_tr = [ps("ps_tr0", [H, EG, 128], f32), ps("ps_tr1", [H, EG, 128], f32)]
    ps_o = [ps("ps_o0", [128, EG, O], f32), ps("ps_o1", [128, EG, O], f32)]

    s_in = sem("s_in")
    s_x = [sem("s_x0"), sem("s_x1")]
    s_mm = sem("s_mm")
    s_ppf = sem("s_ppf")
    s_ep = sem("s_ep")
    s_gw = sem("s_gw")
    s_cc = sem("s_cc")
    s_gd = [sem(f"s_gd{i}") for i in range(NS)]   # gather landed (16/call)
    s_dv = [sem(f"s_dv{i}") for i in range(NS)]   # DVE tree done (1/call)
    s_pw = [sem(f"s_pw{i}") for i in range(NS)]   # partial written (16/call)
    s_lg = [sem(f"s_lg{i}") for i in range(2 * LS)]  # L2 landed
    s_lc = [sem(f"s_lc{i}") for i in range(2 * LS)]  # L2 consumed
    s_gu = sem("s_gu")
    s_prep = sem("s_prep")
    s_ah = sem("s_ah")
    s_u = sem("s_u")
    s_ur = sem("s_ur")
    s_tr = sem("s_tr")
    s_trc = sem("s_trc")
    s_mo = sem("s_mo")
    s_ob = sem("s_ob")
    s_ow = [sem("s_ow0"), sem("s_ow1")]

    def slot_view(s, nt, D):                  # [128, nt, D, H]
        base = s * SLOT_F32
        return stage[:, base:base + nt * D * H].rearrange(
            "p (n d m) -> p n d m", d=D, m=H)

    def slot_flat(s, ncols):                  # [128, ncols, H] gather target
        base = s * SLOT_F32
        return stage[:, base:base + ncols * H].rearrange(
            "p (n m) -> p n m", m=H)

    def l2_view(r, ls, nt):                   # [128, nt, 2, H]
        base = (r * LS + ls) * TG * 2 * H
        return l2st[:, base:base + nt * 2 * H].rearrange(
            "p (n k m) -> p n k m", k=2, m=H)

    def stage_slot_x(s, kt, w):               # [128, kt, w] bf16 xT view (l2st)
        wlen = 768 * kt // 2                  # f32 words per slot (bf16 data)
        return l2st[:, s * wlen:(s + 1) * wlen].bitcast(bf16).rearrange(
            "p (k m) -> p k m", k=kt)[:, :, :w]

    def gsrc(c):                              # gather source slice for chunk c
        gf = gf0 if c < 2 else gf1
        q = c % 2
        return gf[q * cfg.CH:(q + 1) * cfg.CH, :]

    xT3 = xT_h.ap().rearrange("(k p) m -> p k m", p=128)
    w13 = w1_h.ap().rearrange("(k p) m -> p k m", p=128)
    cc0_3 = cc0.ap().rearrange("(n p) m -> p n m", p=128)
    cc1_3 = cc1.ap().rearrange("(n p) m -> p n m", p=128)

    n_in_dmas = 12

    tiles_per_group = [(w + 127) // 128 for w in MGW]
    cum_tiles = np.cumsum([0] + tiles_per_group)

    # ============================== setup block ==============================
    with nc.Block() as blk:
        @blk.sync
        def _(sy):
            sy.dma_start(idxg_sb[:, :], gidx_h[:, :]).then_inc(s_in, 16)
            sy.dma_start(idxl_sb[:, :], lidx_h[:, :]).then_inc(s_in, 16)
            sy.dma_start(w1_sb[:, :, :], w13).then_inc(s_in, 16)
            sy.dma_start(w2_sb[:, :], w2_h[:, :]).then_inc(s_in, 16)
            sy.dma_start(b1r_sb[:, :], b1r_h[:, :]).then_inc(s_in, 16)
            sy.dma_start(b2r_sb[:, :], b2r_h[:, :]).then_inc(s_in, 16)
            sy.dma_start(id_sb[:, :], id_h[:, :]).then_inc(s_in, 16)
            sy.dma_start(dinv_sb[:, :], dinv_h[:, :]).then_inc(s_in, 16)
            sy.dma_start(dinvh_sb[:, :], dinvh_h[:, :]).then_inc(s_in, 16)
            sy.dma_start(dinv2h_sb[:, :], dinv2h_h[:, :]).then_inc(s_in, 16)
            sy.dma_start(rdinv_sb[:, :], rdinv_h[:, :]).then_inc(s_in, 16)
            sy.wait_ge(s_in, 16 * (n_in_dmas - 1))
            col = 0
            for mg, w in enumerate(MGW):
                if mg >= 2:
                    sy.wait_ge(s_mm, int(cum_tiles[mg - 1]))
                sy.dma_start(stage_slot_x(mg % 2, cfg.KT, w),
                             xT3[:, :, col:col + w]).then_inc(s_x[mg % 2], 16)
                col += w
            sy.wait_ge(s_ep, TH)
            sy.dma_start(cc0_3, g_sb[:, 0:TH, :]).then_inc(s_gw, 16)
            sy.wait_ge(s_ep, T)
            sy.dma_start(cc1_3, g_sb[:, TH:T, :]).then_inc(s_gw, 16)
            sy.wait_ge(s_gw, 32)

        @blk.tensor
        def _(pe):
            pe.wait_ge(s_in, 16 * (n_in_dmas - 1))
            ti = 0
            for mg, w in enumerate(MGW):
                pe.wait_ge(s_x[mg % 2], 16 * (mg // 2 + 1))
                nt = tiles_per_group[mg]
                for m in range(nt):
                    mw = min(128, w - m * 128)
                    if ti >= 2:
                        pe.wait_ge(s_ppf, ti - 1)
                    for k in range(cfg.KT):
                        ins = pe.matmul(
                            ps_mm[ti % 2][:mw, :],
                            stage_slot_x(mg % 2, cfg.KT, w)[:, k, m * 128:m * 128 + mw],
                            w1_sb[:, k, :],
                            start=(k == 0), stop=(k == cfg.KT - 1))
                        if k == cfg.KT - 1:
                            ins.then_inc(s_mm)
                    ti += 1

        @blk.vector
        def _(ve):
            ve.wait_ge(s_in, 16 * (n_in_dmas - 1))
            for ti in range(T):
                ve.wait_ge(s_mm, ti + 1)
                if ti >= 2:
                    ve.wait_ge(s_ep, ti - 1)   # Act consumed tmp slot
                ve.tensor_add(tmp_sb[:, ti % 2, :], ps_mm[ti % 2][:, :],
                              b1r_sb[:, :]).then_inc(s_ppf)

        @blk.scalar
        def _(ac):
            ac.wait_ge(s_in, 16 * (n_in_dmas - 1))
            for ti in range(T):
                ac.wait_ge(s_ppf, ti + 1)
                ac.activation(g0a_sb[:, ti, :], tmp_sb[:, ti % 2, :],
                              mybir.ActivationFunctionType.Copy,
                              scale=dinvh_sb[:, ti:ti + 1])
                ac.activation(g_sb[:, ti, :], tmp_sb[:, ti % 2, :],
                              mybir.ActivationFunctionType.Copy,
                              scale=dinv_sb[:, ti:ti + 1]).then_inc(s_ep)

        @blk.gpsimd
        def _(gp):
            gp.load_library(mlp)
            gp.wait_ge(s_gw, 16)
            gp.collective_compute(
                "AllGather", mybir.AluOpType.bypass, replica_groups=rg,
                ins=[cc0.ap().opt()], outs=[gf0.ap().opt()],
            ).then_inc(s_cc)
            gp.wait_ge(s_gw, 32)
            gp.collective_compute(
                "AllGather", mybir.AluOpType.bypass, replica_groups=rg,
                ins=[cc1.ap().opt()], outs=[gf1.ap().opt()],
            ).then_inc(s_cc)

    # ============================ iteration blocks ===========================
    for t in range(cfg.K):
        last = (t == cfg.K - 1)
        with nc.Block() as blk:
            @blk.gpsimd
            def _(gp, t=t, last=last):
                gp.wait_ge(s_cc, 2 * t + 1)
                bar_done = [False, False]
                for kind, k in ISSUE:
                    if kind == "m":
                        (c, coff, ncols, D, nt, t0), s, gdp, dvp, pwp = MAIN[t][k]
                        if k == nA:
                            gp.wait_ge(s_cc, 2 * t + 2)
                        if pwp > 0:
                            gp.wait_ge(s_pw[s], 16 * pwp)
                        nb = ncols * 128
                        off = coff * 128
                        gp.dma_gather(
                            slot_flat(s, ncols),
                            gsrc(c),
                            idxg_sb[:, off // 16:(off + nb) // 16],
                            nb, nb, H, elem_step=H, queue_num=k % NQ,
                            single_packet=SP,
                        ).then_inc(s_gd[s], 16)
                    else:
                        (r, t0, nt, loff), sl, lgp = L2S[t][k]
                        if not bar_done[r]:
                            bs, btgt = (BARA[t] if r == 0 else BARB[t])
                            gp.wait_ge(s_pw[bs], 16 * btgt)
                            bar_done[r] = True
                        if lgp > 0:
                            gp.wait_ge(s_lc[sl], lgp)
                        nb = nt * 2 * 128
                        gp.dma_gather(
                            l2_view(r, sl - r * LS, nt).rearrange(
                                "p n k m -> p (n k) m"),
                            partials[2 * r * PL:(2 * r + 2) * PL, :],
                            idxl_sb[:, loff // 16:(loff + nb) // 16],
                            nb, nb, H, elem_step=H, queue_num=k % NQ,
                            single_packet=SP,
                        ).then_inc(s_lg[sl], 16)
                if not last:
                    gp.wait_ge(s_gw, 16 * (2 * t + 3))
                    gp.collective_compute(
                        "AllGather", mybir.AluOpType.bypass, replica_groups=rg,
                        ins=[cc0.ap().opt()], outs=[gf0.ap().opt()],
                    ).then_inc(s_cc)
                    gp.wait_ge(s_gw, 16 * (2 * t + 4))
                    gp.collective_compute(
                        "AllGather", mybir.AluOpType.bypass, replica_groups=rg,
                        ins=[cc1.ap().opt()], outs=[gf1.ap().opt()],
                    ).then_inc(s_cc)

            @blk.vector
            def _(ve, t=t, last=last):
                # trees + level-2 combines in the unified issue order
                for kind, k in ISSUE:
                    if kind == "m":
                        (c, coff, ncols, D, nt, t0), s, gdp, dvp, pwp = MAIN[t][k]
                        ve.wait_ge(s_gd[s], 16 * (gdp + 1))
                        if D >= 2:
                            v = slot_view(s, nt, D)
                            cur = D
                            ins = None
                            while cur > 1:
                                h = cur // 2
                                lo = cur - h
                                ins = ve.tensor_add(v[:, :, 0:h, :],
                                                    v[:, :, 0:h, :],
                                                    v[:, :, lo:lo + h, :])
                                cur = lo
                            ins.then_inc(s_dv[s])
                        continue
                    (r, t0, nt, loff), sl, lgp = L2S[t][k]
                    ve.wait_ge(s_lg[sl], 16 * (lgp + 1))
                    v = l2_view(r, sl - r * LS, nt)
                    mr = msum_sb[:, t0:t0 + nt, :]
                    if r == 0:
                        ve.tensor_add(mr, v[:, :, 0, :],
                                      v[:, :, 1, :]).then_inc(s_lc[sl])
                    else:
                        ve.tensor_add(mr, mr, v[:, :, 0, :])
                        ve.tensor_add(mr, mr, v[:, :, 1, :]).then_inc(s_lc[sl])
                        ins = ve.tensor_add(mr, mr, g_sb[:, t0:t0 + nt, :])
                        if not last:
                            for ti in range(t0, t0 + nt):
                                ins = ve.scalar_tensor_tensor(
                                    g_sb[:, ti, :], msum_sb[:, ti, :],
                                    dinv2h_sb[:, ti:ti + 1], g0a_sb[:, ti, :],
                                    mybir.AluOpType.mult, mybir.AluOpType.add)
                                if ti == TH - 1 or ti == T - 1:
                                    ins.then_inc(s_gu)
                        else:
                            if t0 + nt == TH or t0 + nt == T:
                                ins.then_inc(s_gu)

            @blk.sync
            def _(sy, t=t, last=last):
                # protect previous iteration's L2 reads of `partials`
                if t >= 1:
                    sy.wait_ge(s_gu, 2 * t)
                # partial write-outs (issue order; HWDGE FIFO per engine)
                for (c, coff, ncols, D, nt, t0), s, gdp, dvp, pwp in MAIN[t]:
                    if D >= 2:
                        sy.wait_ge(s_dv[s], dvp + 1)
                    else:
                        sy.wait_ge(s_gd[s], 16 * (gdp + 1))
                    v = slot_view(s, nt, D)
                    dst = partials[c * PL + t0 * 128:
                                   c * PL + (t0 + nt) * 128, :].rearrange(
                                       "(n p) m -> p n m", p=128)
                    sy.dma_start(dst, v[:, :, 0, :]).then_inc(s_pw[s], 16)
                if not last:
                    sy.wait_ge(s_gu, 2 * t + 1)
                    sy.dma_start(cc0_3, g_sb[:, 0:TH, :]).then_inc(s_gw, 16)
                    sy.wait_ge(s_gu, 2 * t + 2)
                    sy.dma_start(cc1_3, g_sb[:, TH:T, :]).then_inc(s_gw, 16)
                    sy.wait_ge(s_gw, 16 * (2 * t + 4))

    # ============================== epilogue =================================
    # batched by EG-tile groups: per-engine ops run back-to-back inside a
    # group, semaphores hop once per group instead of once per tile.
    with nc.Block() as blk:
        @blk.scalar
        def _(ac):
            for g in range(NGE):
                sl = g % 2
                if g >= 1:
                    ac.wait_ge(s_u, g)          # DVE consumed ah (single buf)
                for i in range(EG):
                    ti = g * EG + i
                    ins = ac.activation(ah_sb[:, i, :], g0a_sb[:, ti, :],
                                        mybir.ActivationFunctionType.Copy,
                                        scale=rdinv_sb[:, ti:ti + 1])
                ins.then_inc(s_ah)
                ac.wait_ge(s_u, g + 1)
                if g >= 2:
                    ac.wait_ge(s_tr, g - 1)     # PE consumed ur slot
                ac.activation(
                    ur_sb[:, sl, :, :].rearrange("p n m -> p (n m)"),
                    u_sb[:, sl, :, :].rearrange("p n m -> p (n m)"),
                    mybir.ActivationFunctionType.Relu).then_inc(s_ur)

        @blk.vector
        def _(ve):
            ve.wait_ge(s_gu, 2 * cfg.K - 1)
            for g in range(NGE):
                sl = g % 2
                if g * EG == TH:
                    ve.wait_ge(s_gu, 2 * cfg.K)
                ve.wait_ge(s_ah, g + 1)
                if g >= 2:
                    ve.wait_ge(s_ur, g - 1)     # Act consumed u slot
                for i in range(EG):
                    ti = g * EG + i
                    ins = ve.scalar_tensor_tensor(
                        u_sb[:, sl, i, :], msum_sb[:, ti, :],
                        dinvh_sb[:, ti:ti + 1], ah_sb[:, i, :],
                        mybir.AluOpType.mult, mybir.AluOpType.add)
                ins.then_inc(s_u)
                ve.wait_ge(s_tr, g + 1)
                if g >= 1:
                    ve.wait_ge(s_mo, g)         # PE consumed lhsT (single buf)
                ve.tensor_copy(
                    lhsT_sb[:, :, :].rearrange("p n m -> p (n m)"),
                    ps_tr[sl][:, :, :].rearrange("p n m -> p (n m)"),
                ).then_inc(s_trc)
                ve.wait_ge(s_mo, g + 1)
                if g >= 2:
                    ve.wait_ge(s_ow[sl], 16 * (g // 2))
                for i in range(EG):
                    ins = ve.tensor_add(out_sb[:, sl, i, :],
                                        ps_o[sl][:, i, :], b2r_sb[:, :])
                ins.then_inc(s_ob)

        @blk.tensor
        def _(pe):
            for g in range(NGE):
                sl = g % 2
                pe.wait_ge(s_ur, g + 1)
                if g >= 2:
                    pe.wait_ge(s_trc, g - 1)    # DVE consumed ps_tr slot
                for i in range(EG):
                    ins = pe.transpose(ps_tr[sl][:, i, :], ur_sb[:, sl, i, :],
                                       id_sb[:, :])
                ins.then_inc(s_tr)
                pe.wait_ge(s_trc, g + 1)
                if g >= 2:
                    pe.wait_ge(s_ob, g - 1)     # DVE consumed ps_o slot
                for i in range(EG):
                    ins = pe.matmul(ps_o[sl][:, i, :], lhsT_sb[:, i, :],
                                    w2_sb[:, :], start=True, stop=True)
                ins.then_inc(s_mo)

        @blk.sync
        def _(sy):
            for g in range(NGE):
                sy.wait_ge(s_ob, g + 1)
                dst = out_h[g * EG:(g + 1) * EG, :, :].rearrange("n p m -> p n m")
                sy.dma_start(dst, out_sb[:, g % 2, :, :]).then_inc(
                    s_ow[g % 2], 16)
            for par in range(2):
                n_par = (NGE + 1 - par) // 2
                if n_par:
                    sy.wait_ge(s_ow[par], 16 * n_par)

    print(f"SBUF used: {(nc.sbuf_base + (nc.SBUF_PARTITION_SIZE_BYTES - nc.sbuf_top)) / 1024:.0f} KB/part "
          f"(base {nc.sbuf_base//1024}KB top-res {(nc.SBUF_PARTITION_SIZE_BYTES - nc.sbuf_top)//1024}KB of {nc.SBUF_PARTITION_SIZE_BYTES//1024}KB) "
          f"NC={NC} NL={NL} S1={S1} pad={plan.pad_frac:.3f} maxD={plan.maxD}")
    if compile_for_hw:
        nc.compile()
    return nc


# ----------------------------------------------------------------- kernel()

_CACHE = {}


def _run(cfg, inputs, trace=False):
    from concourse.bass_utils import run_bass_kernel_spmd

    in_maps, plan = host_prep(cfg, inputs["x"], inputs["edge_index"],
                              inputs["W1"], inputs["b1"],
                              inputs["W2"], inputs["b2"])
    key = (cfg.N, cfg.E, plan.S1, tuple(b[:4] for b in plan.calls))
    if key not in _CACHE:
        _CACHE[key] = build_graph(cfg, plan)
    nc = _CACHE[key]
    res = run_bass_kernel_spmd(nc, in_maps, list(range(cfg.cores)), trace=trace)
    ph = _phys_map(cfg)
    outs = []
    for c in range(cfg.cores):
        o = np.asarray(res.results[c]["out"]).reshape(cfg.PL, cfg.O)
        outs.append(o[ph])
    return np.concatenate(outs, axis=0), res


def kernel(**inputs):
    out, _ = _run(FULL, inputs)
    return out
